# revision 4
# baseline (speedup 1.0000x reference)
"""Trainium2 Bass kernel for nn_ConcatAttention.

Reference computation (S=2048, B=32, E=H=1024):
    he = h_enc transposed to (B,S,E); ht = h_t transposed to (B,1,H)
    compr_he = he @ W_enc.T            (B,S,H)
    compr_ht = ht @ W_dec.T            (B,1,H)
    scores   = tanh((compr_he + compr_ht) @ w_score.T)   (B,S,1)
    alphas   = softmax(scores, axis=1)
    context  = sum_s(he * alphas)[None]                  (1,B,E)
    returns (context, alphas)

Algebraic collapse: w_score @ (x @ W.T).T == x @ (W.T @ w_score), so with
    v = W_enc.T @ w_score   (E,)
    u = W_dec.T @ w_score   (H,)
    scores[b,s] = tanh(he[b,s,:] . v + u . ht[b])
the big (B,S,E)@(E,H) projection disappears and the kernel is a single
memory-bound pass over h_enc. tanh bounds scores to (-1,1) so softmax
needs no max subtraction; context accumulates with unnormalized exp
weights on the PE and is scaled by 1/Z at the end.

Sharding: data-parallel over batch B across 8 cores (4 batches/core);
weights replicated. No collectives.

Per-core layout: s = p*16 + t (p = SBUF partition, t = s-tile index).
Each s-tile DMA brings [128, 4*1024] (all 4 local batches) = 2 MiB with
16 KiB contiguous per partition. Scores via fused DVE tensor_tensor_reduce
against a broadcast v; context via M=1 PE matmuls (exp column stationary).
"""

import numpy as np

import concourse.bass as bass
import concourse.tile as tile
from concourse import bacc, mybir
from concourse.bass_utils import run_bass_kernel_spmd

S, B, E, H = 2048, 32, 1024, 1024
NCORES = 8
BL = B // NCORES  # 4 batches per core
T = 16            # s-tiles; s = p*T + t
F32 = mybir.dt.float32
AF = mybir.ActivationFunctionType
ALU = mybir.AluOpType

_cached_nc = None


def _build_nc():
    nc = bacc.Bacc("TRN2", target_bir_lowering=False, debug=False, num_devices=NCORES)
    h_enc = nc.dram_tensor("h_enc", [S, BL, E], F32, kind="ExternalInput").ap()
    h_t = nc.dram_tensor("h_t", [1, BL, H], F32, kind="ExternalInput").ap()
    W_enc = nc.dram_tensor("W_enc", [H, E], F32, kind="ExternalInput").ap()
    W_dec = nc.dram_tensor("W_dec", [H, H], F32, kind="ExternalInput").ap()
    w_score = nc.dram_tensor("w_score", [1, H], F32, kind="ExternalInput").ap()
    context = nc.dram_tensor("context", [1, BL, E], F32, kind="ExternalOutput").ap()
    alphas = nc.dram_tensor("alphas", [BL, S, 1], F32, kind="ExternalOutput").ap()

    with tile.TileContext(nc) as tc, \
         tc.tile_pool(name="singles", bufs=1) as singles, \
         tc.tile_pool(name="wpool", bufs=3) as wpool, \
         tc.tile_pool(name="hepool", bufs=6) as hepool, \
         tc.tile_pool(name="scrpool", bufs=3) as scrpool, \
         tc.tile_pool(name="dotpool", bufs=3) as dotpool, \
         tc.tile_pool(name="psum", bufs=1, space="PSUM") as psum:

        ones = singles.tile([128, 128], F32)
        nc.vector.memset(ones, 1.0)

        # w as columns: w_col[p, j] = w_score[0, j*128 + p]
        w_col = singles.tile([128, 8], F32)
        nc.sync.dma_start(out=w_col, in_=w_score.rearrange("o (j p) -> p (j o)", p=128))

        ht_sb = singles.tile([BL, H], F32)
        nc.sync.dma_start(out=ht_sb, in_=h_t[0])

        # ---- v = W_enc.T @ w, u = W_dec.T @ w : accumulate [1,512] halves in PSUM
        v_ps0 = psum.tile([1, 512], F32)
        v_ps1 = psum.tile([1, 512], F32)
        u_ps0 = psum.tile([1, 512], F32)
        u_ps1 = psum.tile([1, 512], F32)
        for k in range(8):
            we_t = wpool.tile([128, E], F32, tag="wt")
            nc.sync.dma_start(out=we_t, in_=W_enc[k * 128:(k + 1) * 128, :])
            nc.tensor.matmul(v_ps0, w_col[:, k:k + 1], we_t[:, 0:512],
                             start=(k == 0), stop=(k == 7))
            nc.tensor.matmul(v_ps1, w_col[:, k:k + 1], we_t[:, 512:1024],
                             start=(k == 0), stop=(k == 7))
        for k in range(8):
            wd_t = wpool.tile([128, H], F32, tag="wt")
            nc.sync.dma_start(out=wd_t, in_=W_dec[k * 128:(k + 1) * 128, :])
            nc.tensor.matmul(u_ps0, w_col[:, k:k + 1], wd_t[:, 0:512],
                             start=(k == 0), stop=(k == 7))
            nc.tensor.matmul(u_ps1, w_col[:, k:k + 1], wd_t[:, 512:1024],
                             start=(k == 0), stop=(k == 7))

        v_sb = singles.tile([1, E], F32)
        nc.vector.tensor_copy(v_sb[:, 0:512], v_ps0)
        nc.vector.tensor_copy(v_sb[:, 512:1024], v_ps1)
        u_sb = singles.tile([1, H], F32)
        nc.vector.tensor_copy(u_sb[:, 0:512], u_ps0)
        nc.vector.tensor_copy(u_sb[:, 512:1024], u_ps1)

        # ---- broadcast v across all 128 partitions (ones outer-product on PE)
        v_bcast = singles.tile([128, E], F32)
        for h in range(2):
            vb_ps = psum.tile([128, 512], F32, tag="tiny_ps", bufs=2)
            nc.tensor.matmul(vb_ps, ones[0:1, :], v_sb[0:1, h * 512:(h + 1) * 512])
            nc.vector.tensor_copy(v_bcast[:, h * 512:(h + 1) * 512], vb_ps)

        # ---- u broadcast to BL partitions; c_b = u . ht_b
        u_b4 = singles.tile([BL, H], F32)
        for h in range(2):
            ub_ps = psum.tile([BL, 512], F32, tag="tiny_ps", bufs=2)
            nc.tensor.matmul(ub_ps, ones[0:1, 0:BL], u_sb[0:1, h * 512:(h + 1) * 512])
            nc.vector.tensor_copy(u_b4[:, h * 512:(h + 1) * 512], ub_ps)
        scr4 = singles.tile([BL, H], F32)
        c4 = singles.tile([BL, 1], F32)
        nc.vector.tensor_mul(scr4, ht_sb, u_b4)
        nc.vector.reduce_sum(out=c4, in_=scr4, axis=mybir.AxisListType.X)
        # c4 [BL,1] (partitions 0..3) -> c_row [1,BL] on partition 0
        c_row = singles.tile([1, BL], F32)
        nc.sync.dma_start(out=c_row, in_=c4)
        cb_ps = psum.tile([128, BL], F32, tag="tiny_ps", bufs=2)
        nc.tensor.matmul(cb_ps, ones[0:1, :], c_row[0:1, :])
        cb_all = singles.tile([128, BL], F32)
        nc.vector.tensor_copy(cb_all, cb_ps)

        # ---- main streaming loop over s-tiles
        exp_all = singles.tile([128, BL * T], F32)  # col = b*T + t
        he_view = h_enc.rearrange("(p t) b e -> t p (b e)", t=T)
        ctx_ps = [psum.tile([128, 512], F32, name=f"ctx_ps{h}") for h in range(2)]
        for t in range(T):
            he_t = hepool.tile([128, BL * E], F32, tag="he")
            nc.sync.dma_start(out=he_t, in_=he_view[t])
            dots4 = dotpool.tile([128, BL], F32, tag="dots")
            for b in range(BL):
                # DVE: product; ScalarE: free-dim reduce via Identity+accum
                scr = scrpool.tile([128, E], F32, tag="scr")
                nc.vector.tensor_mul(scr, he_t[:, b * E:(b + 1) * E], v_bcast)
                scr2 = scrpool.tile([128, E], F32, tag="scr2")
                nc.scalar.activation(scr2, scr, AF.Identity,
                                     accum_out=dots4[:, b:b + 1])
            dotsc = dotpool.tile([128, BL], F32, tag="dotsc")
            nc.vector.tensor_add(dotsc, dots4, cb_all)
            tanh4 = dotpool.tile([128, BL], F32, tag="tanh")
            nc.scalar.activation(tanh4, dotsc, AF.Tanh)
            exp_t = exp_all.rearrange("p (b t) -> p t b", t=T)[:, t, :]
            nc.scalar.activation(exp_t, tanh4, AF.Exp)
            for b in range(BL):
                w_ap = exp_all[:, b * T + t: b * T + t + 1]
                for h in range(2):
                    nc.tensor.matmul(
                        ctx_ps[h][32 * b:32 * b + 1, :], w_ap,
                        he_t[:, b * E + h * 512: b * E + (h + 1) * 512],
                        start=(t == 0), stop=(t == T - 1),
                        tile_position=(0, 32 * b))

        # ---- softmax normalization
        zred = singles.tile([128, BL], F32)
        for b in range(BL):
            nc.vector.reduce_sum(out=zred[:, b:b + 1],
                                 in_=exp_all[:, b * T:(b + 1) * T],
                                 axis=mybir.AxisListType.X)
        z_ps = psum.tile([1, BL], F32, tag="tiny_ps", bufs=2)
        nc.tensor.matmul(z_ps, ones[:, 0:1], zred)
        rz_row = singles.tile([1, BL], F32)
        nc.vector.reciprocal(rz_row, z_ps)
        rz_ps = psum.tile([128, BL], F32, tag="tiny_ps", bufs=2)
        nc.tensor.matmul(rz_ps, ones[0:1, :], rz_row[0:1, :])
        rz_all = singles.tile([128, BL], F32)
        nc.vector.tensor_copy(rz_all, rz_ps)

        alphas_sb = singles.tile([128, BL * T], F32)
        for b in range(BL):
            nc.vector.tensor_scalar_mul(alphas_sb[:, b * T:(b + 1) * T],
                                        exp_all[:, b * T:(b + 1) * T],
                                        rz_all[:, b:b + 1])
        nc.sync.dma_start(
            out=alphas.rearrange("b (p t) o -> p b (t o)", t=T),
            in_=alphas_sb.rearrange("p (b t) -> p b t", t=T))

        # context: scale by 1/Z while copying PSUM->SBUF, then one DMA out
        ctx_stage = singles.tile([128, E], F32)
        for b in range(BL):
            for h in range(2):
                nc.scalar.activation(
                    out=ctx_stage[32 * b:32 * b + 1, h * 512:(h + 1) * 512],
                    in_=ctx_ps[h][32 * b:32 * b + 1, :],
                    func=AF.Copy, scale=rz_all[32 * b:32 * b + 1, b:b + 1])
        nc.sync.dma_start(
            out=context[0],
            in_=ctx_stage.rearrange("(a c) e -> a c e", c=32)[:, 0, :])

    nc.compile()
    return nc


def _get_nc():
    global _cached_nc
    if _cached_nc is None:
        _cached_nc = _build_nc()
    return _cached_nc


def _shard_inputs(h_t, h_enc, W_enc, W_dec, w_score):
    in_maps = []
    for i in range(NCORES):
        sl = slice(i * BL, (i + 1) * BL)
        in_maps.append({
            "h_enc": np.ascontiguousarray(h_enc[:, sl, :], dtype=np.float32),
            "h_t": np.ascontiguousarray(h_t[:, sl, :], dtype=np.float32),
            "W_enc": np.ascontiguousarray(W_enc, dtype=np.float32),
            "W_dec": np.ascontiguousarray(W_dec, dtype=np.float32),
            "w_score": np.ascontiguousarray(w_score, dtype=np.float32),
        })
    return in_maps


def run(h_t, h_enc, W_enc, W_dec, w_score, **run_kwargs):
    nc = _get_nc()
    in_maps = _shard_inputs(h_t, h_enc, W_enc, W_dec, w_score)
    res = run_bass_kernel_spmd(nc, in_maps, core_ids=list(range(NCORES)),
                               **run_kwargs)
    context = np.concatenate([r["context"] for r in res.results], axis=1)
    alphas = np.concatenate([r["alphas"] for r in res.results], axis=0)
    return (context, alphas), res


def kernel(h_t, h_enc, W_enc, W_dec, w_score):
    (context, alphas), _ = run(h_t, h_enc, W_enc, W_dec, w_score)
    return (context, alphas)


# revision 8
# speedup vs baseline: 1.1696x; 1.1696x over previous
"""Trainium2 Bass kernel for nn_ConcatAttention.

Reference computation (S=2048, B=32, E=H=1024):
    he = h_enc transposed to (B,S,E); ht = h_t transposed to (B,1,H)
    compr_he = he @ W_enc.T            (B,S,H)
    compr_ht = ht @ W_dec.T            (B,1,H)
    scores   = tanh((compr_he + compr_ht) @ w_score.T)   (B,S,1)
    alphas   = softmax(scores, axis=1)
    context  = sum_s(he * alphas)[None]                  (1,B,E)
    returns (context, alphas)

Algebraic collapse: w_score @ (x @ W.T).T == x @ (W.T @ w_score), so with
    v = W_enc.T @ w_score   (E,)
    u = W_dec.T @ w_score   (H,)
    scores[b,s] = tanh(he[b,s,:] . v + u . ht[b])
the big (B,S,E)@(E,H) projection disappears and the kernel is a single
memory-bound pass over h_enc. tanh bounds scores to (-1,1) so softmax
needs no max subtraction; context accumulates with unnormalized exp
weights on the PE and is scaled by 1/Z at the end.

Sharding: data-parallel over batch B across 8 cores (4 batches/core);
weights replicated. No collectives.

Per-core layout: s = p*16 + t (p = SBUF partition, t = s-tile index).
Each s-tile DMA brings [128, 4*1024] (all 4 local batches) = 2 MiB with
16 KiB contiguous per partition. Scores: DVE multiplies he by broadcast
v, ScalarE reduces via Identity+accum (two engines split the work).
Context: M=1 PE matmuls (exp column stationary) into per-batch PSUM
col-groups via tile_position.
"""

import numpy as np

import concourse.bass as bass
import concourse.tile as tile
from concourse import bacc, mybir
from concourse.bass_utils import run_bass_kernel_spmd

S, B, E, H = 2048, 32, 1024, 1024
NCORES = 8
BL = B // NCORES  # 4 batches per core
T = 16            # s-tiles; s = p*T + t
F32 = mybir.dt.float32
BF16 = mybir.dt.bfloat16
AF = mybir.ActivationFunctionType
ALU = mybir.AluOpType

# Context matmuls in bf16 (he cast on the otherwise-idle GPSIMD engine):
# fp32 PE matmuls run at 1/4 rate (two half-speed passes), so bf16 cuts the
# context pass from ~110-220us of PE time to ~27-55us. Only `context` takes
# the ~1e-3 precision hit; `alphas` stays full fp32.
CTX_BF16 = True

_cached_nc = None


def _emit_body(nc, tc, pools, aps):
    singles, wpool, hepool, scrpool, dotpool, psum = pools
    h_enc, h_t, W_enc, W_dec, w_score, context, alphas = aps

    ones = singles.tile([128, 128], F32, tag="ones")
    nc.vector.memset(ones, 1.0)

    # w as columns: w_col[p, j] = w_score[0, j*128 + p]
    w_col = singles.tile([128, 8], F32, tag="w_col")
    nc.sync.dma_start(out=w_col, in_=w_score.rearrange("o (j p) -> p (j o)", p=128))

    ht_sb = singles.tile([BL, H], F32, tag="ht_sb")
    nc.sync.dma_start(out=ht_sb, in_=h_t[0])

    # ---- v = W_enc.T @ w, u = W_dec.T @ w : accumulate [1,512] halves in PSUM
    v_ps0 = psum.tile([1, 512], F32, tag="v_ps0")
    v_ps1 = psum.tile([1, 512], F32, tag="v_ps1")
    u_ps0 = psum.tile([1, 512], F32, tag="u_ps0")
    u_ps1 = psum.tile([1, 512], F32, tag="u_ps1")
    for k in range(8):
        we_t = wpool.tile([128, E], F32, tag="wt")
        nc.sync.dma_start(out=we_t, in_=W_enc[k * 128:(k + 1) * 128, :])
        nc.tensor.matmul(v_ps0, w_col[:, k:k + 1], we_t[:, 0:512],
                         start=(k == 0), stop=(k == 7))
        nc.tensor.matmul(v_ps1, w_col[:, k:k + 1], we_t[:, 512:1024],
                         start=(k == 0), stop=(k == 7))
    for k in range(8):
        wd_t = wpool.tile([128, H], F32, tag="wt")
        nc.sync.dma_start(out=wd_t, in_=W_dec[k * 128:(k + 1) * 128, :])
        nc.tensor.matmul(u_ps0, w_col[:, k:k + 1], wd_t[:, 0:512],
                         start=(k == 0), stop=(k == 7))
        nc.tensor.matmul(u_ps1, w_col[:, k:k + 1], wd_t[:, 512:1024],
                         start=(k == 0), stop=(k == 7))

    v_sb = singles.tile([1, E], F32, tag="v_sb")
    nc.vector.tensor_copy(v_sb[:, 0:512], v_ps0)
    nc.vector.tensor_copy(v_sb[:, 512:1024], v_ps1)
    u_sb = singles.tile([1, H], F32, tag="u_sb")
    nc.vector.tensor_copy(u_sb[:, 0:512], u_ps0)
    nc.vector.tensor_copy(u_sb[:, 512:1024], u_ps1)

    # ---- broadcast v across all 128 partitions (ones outer-product on PE)
    v_bcast = singles.tile([128, E], F32, tag="v_bcast")
    for h in range(2):
        vb_ps = psum.tile([128, 512], F32, tag="tiny_ps", bufs=2)
        nc.tensor.matmul(vb_ps, ones[0:1, :], v_sb[0:1, h * 512:(h + 1) * 512])
        nc.vector.tensor_copy(v_bcast[:, h * 512:(h + 1) * 512], vb_ps)

    # ---- u broadcast to BL partitions; c_b = u . ht_b
    u_b4 = singles.tile([BL, H], F32, tag="u_b4")
    for h in range(2):
        ub_ps = psum.tile([BL, 512], F32, tag="tiny_ps", bufs=2)
        nc.tensor.matmul(ub_ps, ones[0:1, 0:BL], u_sb[0:1, h * 512:(h + 1) * 512])
        nc.vector.tensor_copy(u_b4[:, h * 512:(h + 1) * 512], ub_ps)
    scr4 = singles.tile([BL, H], F32, tag="scr4")
    c4 = singles.tile([BL, 1], F32, tag="c4")
    nc.vector.tensor_mul(scr4, ht_sb, u_b4)
    nc.vector.reduce_sum(out=c4, in_=scr4, axis=mybir.AxisListType.X)
    # c4 [BL,1] (partitions 0..3) -> c_row [1,BL] on partition 0
    c_row = singles.tile([1, BL], F32, tag="c_row")
    nc.sync.dma_start(out=c_row, in_=c4)
    cb_ps = psum.tile([128, BL], F32, tag="tiny_ps", bufs=2)
    nc.tensor.matmul(cb_ps, ones[0:1, :], c_row[0:1, :])
    cb_all = singles.tile([128, BL], F32, tag="cb_all")
    nc.vector.tensor_copy(cb_all, cb_ps)

    # ---- main streaming loop over s-tiles
    exp_all = singles.tile([128, BL * T], F32, tag="exp_all")  # col = b*T + t
    if CTX_BF16:
        exp_bf_all = singles.tile([128, BL * T], BF16, tag="exp_bf_all")
    he_view = h_enc.rearrange("(p t) b e -> t p (b e)", t=T)
    ctx_ps = [psum.tile([128, 512], F32, name=f"ctx_ps{h}", tag=f"ctx_ps{h}")
              for h in range(2)]
    for t in range(T):
        he_t = hepool.tile([128, BL * E], F32, tag="he")
        nc.sync.dma_start(out=he_t, in_=he_view[t])
        if CTX_BF16:
            he_bf = hepool.tile([128, BL * E], BF16, tag="he_bf", bufs=3)
            nc.gpsimd.tensor_copy(he_bf, he_t)
        dots4 = dotpool.tile([128, BL], F32, tag="dots")
        for b in range(BL):
            # DVE: product; ScalarE: free-dim reduce via Identity+accum
            scr = scrpool.tile([128, E], F32, tag="scr")
            nc.vector.tensor_mul(scr, he_t[:, b * E:(b + 1) * E], v_bcast)
            scr2 = scrpool.tile([128, E], F32, tag="scr2")
            nc.scalar.activation(scr2, scr, AF.Identity,
                                 accum_out=dots4[:, b:b + 1])
        dotsc = dotpool.tile([128, BL], F32, tag="dotsc")
        nc.vector.tensor_add(dotsc, dots4, cb_all)
        tanh4 = dotpool.tile([128, BL], F32, tag="tanh")
        nc.scalar.activation(tanh4, dotsc, AF.Tanh)
        exp_t = exp_all.rearrange("p (b t) -> p t b", t=T)[:, t, :]
        nc.scalar.activation(exp_t, tanh4, AF.Exp)
        if CTX_BF16:
            exp_bf_t = exp_bf_all.rearrange("p (b t) -> p t b", t=T)[:, t, :]
            nc.gpsimd.tensor_copy(exp_bf_t, exp_t)
            mm_w, mm_he = exp_bf_all, he_bf
        else:
            mm_w, mm_he = exp_all, he_t
        for b in range(BL):
            w_ap = mm_w[:, b * T + t: b * T + t + 1]
            for h in range(2):
                nc.tensor.matmul(
                    ctx_ps[h][32 * b:32 * b + 1, :], w_ap,
                    mm_he[:, b * E + h * 512: b * E + (h + 1) * 512],
                    start=(t == 0), stop=(t == T - 1),
                    tile_position=(0, 32 * b))

    # ---- softmax normalization
    zred = singles.tile([128, BL], F32, tag="zred")
    for b in range(BL):
        nc.vector.reduce_sum(out=zred[:, b:b + 1],
                             in_=exp_all[:, b * T:(b + 1) * T],
                             axis=mybir.AxisListType.X)
    z_ps = psum.tile([1, BL], F32, tag="tiny_ps", bufs=2)
    nc.tensor.matmul(z_ps, ones[:, 0:1], zred)
    rz_row = singles.tile([1, BL], F32, tag="rz_row")
    nc.vector.reciprocal(rz_row, z_ps)
    rz_ps = psum.tile([128, BL], F32, tag="tiny_ps", bufs=2)
    nc.tensor.matmul(rz_ps, ones[0:1, :], rz_row[0:1, :])
    rz_all = singles.tile([128, BL], F32, tag="rz_all")
    nc.vector.tensor_copy(rz_all, rz_ps)

    alphas_sb = singles.tile([128, BL * T], F32, tag="alphas_sb")
    for b in range(BL):
        nc.vector.tensor_scalar_mul(alphas_sb[:, b * T:(b + 1) * T],
                                    exp_all[:, b * T:(b + 1) * T],
                                    rz_all[:, b:b + 1])
    nc.sync.dma_start(
        out=alphas.rearrange("b (p t) o -> p b (t o)", t=T),
        in_=alphas_sb.rearrange("p (b t) -> p b t", t=T))

    # context: scale by 1/Z while copying PSUM->SBUF (DVE), then one DMA out
    ctx_stage = singles.tile([128, E], F32, tag="ctx_stage")
    for b in range(BL):
        for h in range(2):
            nc.vector.tensor_scalar_mul(
                ctx_stage[32 * b:32 * b + 1, h * 512:(h + 1) * 512],
                ctx_ps[h][32 * b:32 * b + 1, :],
                rz_all[32 * b:32 * b + 1, b:b + 1])
    nc.sync.dma_start(
        out=context[0],
        in_=ctx_stage.rearrange("(a c) e -> a c e", c=32)[:, 0, :])


def _build_nc(reps=1):
    nc = bacc.Bacc("TRN2", target_bir_lowering=False, debug=False, num_devices=NCORES)
    h_enc = nc.dram_tensor("h_enc", [S, BL, E], F32, kind="ExternalInput").ap()
    h_t = nc.dram_tensor("h_t", [1, BL, H], F32, kind="ExternalInput").ap()
    W_enc = nc.dram_tensor("W_enc", [H, E], F32, kind="ExternalInput").ap()
    W_dec = nc.dram_tensor("W_dec", [H, H], F32, kind="ExternalInput").ap()
    w_score = nc.dram_tensor("w_score", [1, H], F32, kind="ExternalInput").ap()
    context = nc.dram_tensor("context", [1, BL, E], F32, kind="ExternalOutput").ap()
    alphas = nc.dram_tensor("alphas", [BL, S, 1], F32, kind="ExternalOutput").ap()
    aps = (h_enc, h_t, W_enc, W_dec, w_score, context, alphas)

    with tile.TileContext(nc) as tc, \
         tc.tile_pool(name="singles", bufs=1) as singles, \
         tc.tile_pool(name="wpool", bufs=3) as wpool, \
         tc.tile_pool(name="hepool", bufs=6) as hepool, \
         tc.tile_pool(name="scrpool", bufs=3) as scrpool, \
         tc.tile_pool(name="dotpool", bufs=3) as dotpool, \
         tc.tile_pool(name="psum", bufs=1, space="PSUM") as psum:
        pools = (singles, wpool, hepool, scrpool, dotpool, psum)
        for _ in range(reps):
            _emit_body(nc, tc, pools, aps)

    nc.compile()
    return nc


def _get_nc():
    global _cached_nc
    if _cached_nc is None:
        _cached_nc = _build_nc()
    return _cached_nc


def _shard_inputs(h_t, h_enc, W_enc, W_dec, w_score):
    in_maps = []
    for i in range(NCORES):
        sl = slice(i * BL, (i + 1) * BL)
        in_maps.append({
            "h_enc": np.ascontiguousarray(h_enc[:, sl, :], dtype=np.float32),
            "h_t": np.ascontiguousarray(h_t[:, sl, :], dtype=np.float32),
            "W_enc": np.ascontiguousarray(W_enc, dtype=np.float32),
            "W_dec": np.ascontiguousarray(W_dec, dtype=np.float32),
            "w_score": np.ascontiguousarray(w_score, dtype=np.float32),
        })
    return in_maps


def run(h_t, h_enc, W_enc, W_dec, w_score, **run_kwargs):
    nc = _get_nc()
    in_maps = _shard_inputs(h_t, h_enc, W_enc, W_dec, w_score)
    res = run_bass_kernel_spmd(nc, in_maps, core_ids=list(range(NCORES)),
                               **run_kwargs)
    context = np.concatenate([r["context"] for r in res.results], axis=1)
    alphas = np.concatenate([r["alphas"] for r in res.results], axis=0)
    return (context, alphas), res


def kernel(h_t, h_enc, W_enc, W_dec, w_score):
    (context, alphas), _ = run(h_t, h_enc, W_enc, W_dec, w_score)
    return (context, alphas)
